# revision 17
# baseline (speedup 1.0000x reference)
"""Trainium2 Bass kernel for nn_AbsoluteMinimalBlock (rmsnorm -> rank-1 SSM scan -> rmsnorm -> rank-2 FFN).

Math: the whole block is a rank-3 update of x:
    out[t,d] = x[t,d] + h[t]*Wout[d] + g0[t]*W20[d] + g1[t]*W21[d]
  driven by 5 per-token reductions over D:
    d1 = x@(nw*W_in), dW2 = x@(2*Wout), dA = x@(nw*w1_0), dB = x@(nw*w1_1), S0 = sum(x^2)
  with rstd1 = 1/sqrt(S0/D+eps); u = d1*rstd1; h = scan(a, u);
  D*ms2 = S0 + h*dW2 + h^2*|Wout|^2 (analytic); p_r = (d_r + h*(Wout.W1r))*rstd2;
  g_r = gelu_tanh(p_r).

Device layout: DIM-MAJOR fp16. The host pre-transposes each core's shard to
x^T [1024 d, NTOK tokens] fp16 (prefix tokens first), so the per-token dots
need no on-device data transpose: per 128-token tile,
matmul(lhsT=x^T slice [128d,128t], rhs=weights [128d,4]) -> PSUM [128t, 4],
and S0 rides the same trick against x^2 with a ones column. fp16 I/O halves
HBM traffic (rel-err gate is 2e-2; fp16 lands ~3e-4).

Sharding: 8 cores = 4 batches x 2 sequence halves; second-half cores prepend a
`pre`-tile prefix (tail of first half) so the scan state is exact (a^128 decay
per tile).

Structure (per the sim trace, dma_start costs ~500ns of sequencer time and
every vector/scalar op has ~200ns fixed cost, so ops are coarse):
 - ONE rearranged DMA per chunk each way ([128, 8, T] <-> [1024, T] dim-major).
 - chunk = 4 main tiles (512 tokens); chunk 0 also carries the prefix tiles.
 - squares x^2 split ACT/DVE by slice; 5 dots per tile via PE into [128t, 5].
 - chunk-serial carry handoff via tensor_tensor_scan(initial=srow[:, prev]).
 - post-scan per-token math batched over groups of 2 chunks.
 - residual add split: half the slices DVE tensor_add(psum,x); other half PE
   I@x matmul + ACT copy. Software-pipelined emission (engines execute in
   program order): S1(k) | S2(k-1) | S34(group k-4).
"""
import sys, os
for _p in ("/root/.axon_site/_ro/trn_rl_repo", "/opt/trn_rl_repo"):
    if os.path.isdir(_p) and _p not in sys.path:
        sys.path.append(_p)

import numpy as np
import concourse.bass as bass
import concourse.bacc as bacc
import concourse.mybir as mybir
import concourse.tile as tile
from concourse.bass_utils import run_bass_kernel_spmd

F32 = mybir.dt.float32
F16 = mybir.dt.float16
U32 = mybir.dt.uint32
AF = mybir.ActivationFunctionType
OP = mybir.AluOpType
MAGICF = 1597463007.0         # float(0x5f3759df): quake rsqrt seed constant

N_CORES = 8
B, S, D = 4, 8192, 1024
HALF = S // 2
MAIN_TILES = HALF // 128      # 32
CT = 4                        # main tiles per chunk (512 tokens)
NCH = MAIN_TILES // CT        # 8 chunks; chunk 0 additionally holds the prefix
EPS = 1e-6
DVE_SQ = (0, 1, 2, 3)         # slices whose x^2 runs on DVE (rest: ACT)
DVE_ADD = (0, 1)        # 2-slice units evacuated by DVE tensor_add (rest: PE I@x + ACT copy)

_cache = {}


def build_program(pre: int, reps=None, internal_io=False,
                  parts=("sq", "dots", "scan", "b3")):
    parts = set(parts)
    nt = MAIN_TILES + pre
    NTOK = nt * 128
    # chunk c covers tiles [t0, t1); chunk 0 includes the prefix tiles
    chunks = [(0, pre + CT)] + [(pre + CT * i, pre + CT * (i + 1))
                                for i in range(1, NCH)]
    groups = [(chunks[2 * g][0], chunks[2 * g + 1][1]) for g in range(NCH // 2)]

    nc = bacc.Bacc("TRN2", target_bir_lowering=False, debug=False, num_devices=N_CORES)

    if internal_io:
        xd = nc.dram_tensor("x_int", [D, NTOK], F16, kind="Internal").ap()
        yd = nc.dram_tensor("y_int", [D, HALF], F16, kind="Internal").ap()
        dummy_in = nc.dram_tensor("x_in", [128, 4], F32, kind="ExternalInput").ap()
        dummy_out = nc.dram_tensor("y_out", [128, 4], F32, kind="ExternalOutput").ap()
        need_dummy_io = True
    else:
        xd = nc.dram_tensor("x_in", [D, NTOK], F16, kind="ExternalInput").ap()
        yd = nc.dram_tensor("y_out", [D, HALF], F16, kind="ExternalOutput").ap()
        need_dummy_io = False
    w4d = nc.dram_tensor("w4", [D, 4], F16, kind="ExternalInput").ap()
    w3d = nc.dram_tensor("w3", [3, D], F16, kind="ExternalInput").ap()
    identd = nc.dram_tensor("ident", [128, 128], F16, kind="ExternalInput").ap()
    t128d = nc.dram_tensor("t128", [128, 128], F16, kind="ExternalInput").ap()
    frowd = nc.dram_tensor("frow", [128, 1], F16, kind="ExternalInput").ap()
    apow1d = nc.dram_tensor("apow1", [1, 128], F32, kind="ExternalInput").ap()
    alrowd = nc.dram_tensor("alrow", [1, nt], F32, kind="ExternalInput").ap()
    colsd = nc.dram_tensor("cols3", [128, 3], F32, kind="ExternalInput").ap()
    onesd = nc.dram_tensor("ones", [128, 1], F16, kind="ExternalInput").ap()

    with tile.TileContext(nc) as tc:
        with (
            tc.tile_pool(name="xpool", bufs=1) as xpool,
            tc.tile_pool(name="work", bufs=3) as work,
            tc.tile_pool(name="sq", bufs=3) as sqp,
            tc.tile_pool(name="small", bufs=1) as small,
            tc.tile_pool(name="cst", bufs=1) as cst,
            tc.tile_pool(name="ps", bufs=1, space="PSUM") as psp,
        ):
            # ---- constants ----
            vw4 = cst.tile([128, 8, 4], F16, name="vw4")
            w3s = cst.tile([3, D], F16, name="w3s")
            idents = cst.tile([128, 128], F16, name="idents")
            t128s = cst.tile([128, 128], F16, name="t128s")
            frows = cst.tile([128, 1], F16, name="frows")
            apow1s = cst.tile([1, 128], F32, name="apow1s")
            alrows = cst.tile([1, nt], F32, name="alrows")
            cols3s = cst.tile([128, 3], F32, name="cols3s")
            oness = cst.tile([128, 1], F16, name="oness")
            eps_col = cst.tile([128, 1], F32, name="eps_col")
            nc.sync.dma_start(vw4[:], w4d.rearrange("(k p) q -> p k q", p=128))
            nc.sync.dma_start(w3s[:], w3d[:])
            nc.sync.dma_start(idents[:], identd[:])
            nc.sync.dma_start(t128s[:], t128d[:])
            nc.sync.dma_start(frows[:], frowd[:])
            nc.sync.dma_start(apow1s[:], apow1d[:])
            nc.sync.dma_start(alrows[:], alrowd[:])
            nc.sync.dma_start(cols3s[:], colsd[:])
            nc.sync.dma_start(oness[:], onesd[:])
            nc.vector.memset(eps_col[:], float(EPS))

            # ---- persistent per-token arrays (token-major: partition=t%128,
            # free col=tile index) ----
            d5 = small.tile([128, nt, 5], F16, name="d5")
            u = small.tile([128, nt], F16, name="u")
            srow = small.tile([1, nt + 1], F32, name="srow")
            sca = small.tile([128, nt], F32, name="sca")
            scb = small.tile([128, nt], F32, name="scb")
            rst = small.tile([128, nt], F32, name="rst")
            hg16 = small.tile([128, 3, nt], F16, name="hg16")
            g3 = small.tile([3, MAIN_TILES * 128], F16, name="g3")
            if need_dummy_io:
                dum = small.tile([128, 4], F32, name="dum")
                nc.sync.dma_start(dum[:], dummy_in[:])
                nc.sync.dma_start(dummy_out[:], dum[:])

            def body():
                # one rearranged input DMA per chunk, all up front: nothing
                # blocks them, so the ring streams at HBM rate
                x_tiles = {}
                for ci, (t0, t1) in enumerate(chunks):
                    tc_ = (t1 - t0) * 128
                    xt = xpool.tile([128, 8, tc_], F16, tag=f"x{ci}",
                                    name=f"x{ci}")
                    nc.sync.dma_start(
                        xt[:], xd[:, t0 * 128:t1 * 128]
                        .rearrange("(s p) t -> p s t", p=128))
                    x_tiles[ci] = xt
                nc.vector.memset(srow[:, 0:1], 0.0)

                # ---- phase A (per chunk): squares + per-token dots/S0 ----
                qs = {}
                for ci, (t0, t1) in enumerate(chunks):
                    ntl = t1 - t0
                    tc_ = ntl * 128
                    xt = x_tiles[ci]
                    q = sqp.tile([128, 8, tc_], F16, tag="xsq", name=f"xsq{ci}")
                    qs[ci] = q
                    if "sq" in parts:
                        # split ACT/DVE: on real HW the DVE f16 TT runs at 1x,
                        # so neither engine alone keeps the DMA cadence
                        nc.scalar.activation(q[:, 0:4, :], xt[:, 0:4, :],
                                             AF.Square)
                        nc.vector.tensor_mul(q[:, 4:8, :], xt[:, 4:8, :],
                                             xt[:, 4:8, :])
                    if "dots" not in parts:
                        continue
                    dps = psp.tile([128, ntl, 5], F32, tag="d5t",
                                   name=f"d5t{ci}", bufs=1)
                    for tl in range(ntl):
                        for s in range(8):
                            nc.tensor.matmul(
                                dps[:, tl, 0:4],
                                xt[:, s, tl * 128:(tl + 1) * 128],
                                vw4[:, s, :], start=(s == 0), stop=(s == 7))
                        for s in range(8):
                            nc.tensor.matmul(
                                dps[:, tl, 4:5],
                                q[:, s, tl * 128:(tl + 1) * 128],
                                oness[:], start=(s == 0), stop=(s == 7))
                    nc.scalar.copy(d5[:, t0:t1, :], dps[:])

                if "scan" not in parts or "dots" not in parts:
                    return

                def rsqrt_dve(cs):
                    # rst[:, cs] = 1/sqrt(sca[:, cs]) entirely on DVE (quake
                    # seed via dtype-converting tensor_scalar + 1x newton) so
                    # ACT never loads the Sqrt table (conflicts with Gelu's)
                    nc.vector.tensor_scalar(rst[:, cs].bitcast(U32),
                                            sca[:, cs].bitcast(U32),
                                            -0.5, MAGICF, OP.mult, OP.add)
                    for _ in range(1):
                        nc.vector.tensor_mul(scb[:, cs], rst[:, cs], rst[:, cs])
                        nc.vector.tensor_mul(scb[:, cs], scb[:, cs], sca[:, cs])
                        nc.vector.tensor_scalar(scb[:, cs], scb[:, cs],
                                                -0.5, 1.5, OP.mult, OP.add)
                        nc.vector.tensor_mul(rst[:, cs], rst[:, cs], scb[:, cs])

                # ---- global scan: u, within-tile prefixes, carry, h ----
                loc_ps = psp.tile([128, nt], F32, tag="loc", name="loc_ps",
                                  bufs=1)
                f_ps = psp.tile([1, nt], F32, tag="fps", name="f_ps", bufs=1)
                al = slice(0, nt)
                nc.vector.tensor_scalar(sca[:, al], d5[:, al, 4],
                                        float(D * EPS), None, OP.add)
                rsqrt_dve(al)
                nc.vector.tensor_mul(u[:, al], d5[:, al, 0], rst[:, al])
                nc.tensor.matmul(loc_ps[:, al], t128s[:], u[:, al],
                                 start=True, stop=False)
                nc.tensor.matmul(f_ps[:, al], frows[:], u[:, al],
                                 start=True, stop=True)
                nc.vector.tensor_tensor_scan(
                    srow[:, 1:nt + 1], alrows[:, al], f_ps[:, al],
                    srow[:, 0:1], OP.mult, OP.add)
                nc.tensor.matmul(loc_ps[:, al], apow1s[:], srow[:, 0:nt],
                                 start=False, stop=True)

                # ---- global post-scan per-token math ----
                cs = slice(pre, nt)
                h = loc_ps[:, cs]
                nc.scalar.copy(hg16[:, 0, cs], h)
                nc.vector.tensor_mul(sca[:, cs], h, d5[:, cs, 1])
                nc.vector.tensor_add(sca[:, cs], sca[:, cs], d5[:, cs, 4])
                nc.vector.tensor_mul(scb[:, cs], h, hg16[:, 0, cs])
                nc.vector.scalar_tensor_tensor(sca[:, cs], scb[:, cs],
                                               cols3s[:, 0:1], sca[:, cs],
                                               OP.mult, OP.add)
                nc.vector.tensor_scalar(sca[:, cs], sca[:, cs],
                                        float(D * EPS), None, OP.add)
                rsqrt_dve(cs)
                for r in (0, 1):
                    nc.vector.scalar_tensor_tensor(
                        sca[:, cs], h, cols3s[:, 1 + r:2 + r],
                        d5[:, cs, 2 + r], OP.mult, OP.add)
                    nc.vector.tensor_mul(scb[:, cs], sca[:, cs], rst[:, cs])
                    nc.scalar.activation(hg16[:, 1 + r, cs], scb[:, cs],
                                         AF.Gelu_apprx_tanh)

                if "b3" not in parts:
                    return
                # g3 rows: per-tile transposes, copies per 2 chunks
                for gp in range(MAIN_TILES // 8):
                    gt = psp.tile([3, 8, 128], F16, tag="g3t",
                                  name=f"g3t{gp}", bufs=1)
                    for tl in range(8):
                        nc.tensor.transpose(gt[:, tl, :],
                                            hg16[:, :, pre + gp * 8 + tl],
                                            idents[:])
                    nc.scalar.copy(g3[:, gp * 1024:(gp + 1) * 1024], gt[:])

                # ---- phase B: rank-3 + residual, 2-slice units ----
                for ci, (t0, t1) in enumerate(chunks):
                    b0 = max(t0, pre)
                    tc_ = (t1 - b0) * 128
                    mt0 = (b0 - pre) * 128
                    xoff = (b0 - t0) * 128
                    osb = work.tile([128, 8, tc_], F16, tag="osb",
                                    name=f"osb{ci}")
                    for un in range(4):
                        s0_, s1_ = 2 * un, 2 * un + 1
                        bps = psp.tile([128, 2, tc_], F32, tag="b3",
                                       name=f"b3_{ci}_{un}", bufs=2)
                        pe_add = un not in DVE_ADD
                        for j, s in enumerate((s0_, s1_)):
                            nc.tensor.matmul(bps[:, j, :],
                                             w3s[:, s * 128:(s + 1) * 128],
                                             g3[:, mt0:mt0 + tc_],
                                             start=True, stop=not pe_add)
                            if pe_add:
                                nc.tensor.matmul(
                                    bps[:, j, :], idents[:],
                                    x_tiles[ci][:, s, xoff:xoff + tc_],
                                    start=False, stop=True)
                        xs2 = x_tiles[ci][:, s0_:s0_ + 2, xoff:xoff + tc_]
                        if pe_add:
                            nc.scalar.copy(osb[:, s0_:s0_ + 2, :], bps[:])
                        else:
                            nc.vector.tensor_add(osb[:, s0_:s0_ + 2, :],
                                                 bps[:], xs2)
                    nc.sync.dma_start(
                        yd[:, mt0:mt0 + tc_]
                        .rearrange("(s p) t -> p s t", p=128), osb[:])

            if reps is None:
                body()
            else:
                with tc.For_i(0, reps, 1):
                    body()
    nc.compile()
    return nc


def host_constants(norm_w, W_in, a_log, W_out, ffn_w1, ffn_w2, nt):
    a = 1.0 / (1.0 + np.exp(-np.float64(a_log[0])))
    Wn = (norm_w * W_in[:, 0]).astype(np.float64)
    Wout_row = W_out[0, :].astype(np.float64)
    W10n = (norm_w * ffn_w1[:, 0]).astype(np.float64)
    W11n = (norm_w * ffn_w1[:, 1]).astype(np.float64)
    # sqrt(D) folded into the dot weights: the device computes
    # rstd = rsqrt(S + D*eps) which is 1/sqrt(D) times the true rstd
    sd = np.sqrt(np.float64(D))
    w4 = np.stack([Wn * sd, 2.0 * Wout_row, W10n * sd, W11n * sd],
                  axis=1).astype(np.float16)
    w3 = np.stack([Wout_row, ffn_w2[0, :], ffn_w2[1, :]], axis=0).astype(np.float16)
    km = np.arange(128)
    expo = km[None, :] - km[:, None]
    t128 = np.where(expo >= 0, a ** np.maximum(expo, 0), 0.0).astype(np.float16)
    frow = (a ** (127 - km)).astype(np.float16).reshape(128, 1)
    apow1 = (a ** (km + 1)).astype(np.float32).reshape(1, 128)
    alrow = np.full((1, nt), a ** 128, dtype=np.float32)
    cWW = np.float32(Wout_row @ Wout_row)
    c0 = np.float32(Wout_row @ W10n * sd)
    c1 = np.float32(Wout_row @ W11n * sd)
    cols3 = np.tile(np.array([cWW, c0, c1], dtype=np.float32), (128, 1))
    return dict(w4=w4, w3=w3, ident=np.eye(128, dtype=np.float16), t128=t128,
                frow=frow, apow1=apow1, alrow=alrow, cols3=cols3,
                ones=np.ones((128, 1), np.float16)), a


def pre_tiles_for(a: float) -> int:
    n = int(np.ceil(np.log(1e-9) / (128 * np.log(a))))
    # a=sigmoid(a_log) ~ 0.785 for the reference inputs -> n=1
    return min(max(n, 1), 6)


def in_maps_for(x, consts, pre):
    pre_tok = pre * 128
    maps = []
    for c in range(N_CORES):
        b, j = c // 2, c % 2
        if j == 0:
            prefix = np.zeros((pre_tok, D), np.float32)
        else:
            prefix = x[b, HALF - pre_tok:HALF, :]
        xin = np.concatenate([prefix, x[b, j * HALF:(j + 1) * HALF, :]], axis=0)
        m = {"x_in": np.ascontiguousarray(xin.T.astype(np.float16))}
        m.update(consts)
        maps.append(m)
    return maps


def kernel(x, norm_w, W_in, a_log, W_out, ffn_w1, ffn_w2):
    x = np.asarray(x, dtype=np.float32)
    consts, a = host_constants(np.asarray(norm_w), np.asarray(W_in),
                               np.asarray(a_log), np.asarray(W_out),
                               np.asarray(ffn_w1), np.asarray(ffn_w2), nt=34)
    pre = pre_tiles_for(a)
    nt = MAIN_TILES + pre
    consts["alrow"] = np.full((1, nt), np.float64(a) ** 128, dtype=np.float32)

    key = ("plain", pre)
    if key not in _cache:
        _cache[key] = build_program(pre)
    nc = _cache[key]

    res = run_bass_kernel_spmd(nc, in_maps_for(x, consts, pre),
                               core_ids=list(range(N_CORES)))
    out = np.empty((B, S, D), np.float32)
    for c in range(N_CORES):
        b, j = c // 2, c % 2
        out[b, j * HALF:(j + 1) * HALF, :] = res.results[c]["y_out"].T
    return out
